# revision 1
# baseline (speedup 1.0000x reference)
"""Quantized dense MLP kernel for 8 Trainium2 NeuronCores.

Problem: out = relu(inputs @ ((w_int8 - zero_point) * scale) + b)
  inputs [8192, 2048] f32, w_quantized [2048, 8192] int8,
  scale/zero_point f32 scalars, b [8192] f32 -> out [8192, 8192] f32.

Strategy:
- Data-parallel: shard rows of `inputs` across 8 cores (1024 rows each).
- Zero-point folding: w_int = w_int8 - zero_point (zero_point = -3.0) is a
  small integer, exactly representable in bf16/f32. The scale and bias are
  applied on the ScalarEngine in f32: out = Relu(scale * acc + b).
- The PE matmul runs in float32r (TF32-like, full bf16 throughput at
  N=512): exact weights, ~1e-4 rel err from the reduced-precision x
  streaming. (MM_DTYPE="bfloat16" variant: ~6% faster, ~1.7e-3 rel err.)
- On device each core computes outT[j, i] = sum_k w_int[k, j] * xT[k, i]
  (w tile stationary, xT moving), so the bias b_j is a natural
  per-partition activation bias. The host transposes each core's outT
  back and stacks.
- Per core: x^T stays SBUF-resident; weights stream as G large 2D DMAs
  (the sync engine pays ~600 ns issue per DMA regardless of size, so few
  big transfers); group 0 lands piecewise with x on a parallel HW queue
  so the PE starts within ~16 us.
"""

import numpy as np
import ml_dtypes

import concourse.bass as bass
import concourse.mybir as mybir
import concourse.tile as tile
from concourse import bacc
from concourse.bass_utils import run_bass_kernel_spmd

BF16 = ml_dtypes.bfloat16

# Full problem dims (hardcoded per harness contract).
ROWS, D_IN, UNITS = 8192, 2048, 8192
N_CORES = 8
ROWS_C = ROWS // N_CORES  # rows per core

P = 128         # SBUF partitions
N_SLICE = 512   # moving free dim per matmul (one PSUM bank of f32)


def build_nc(scale: float, d_in: int = D_IN, units: int = UNITS,
             rows_c: int = ROWS_C, mm_dtype: str = "bfloat16"):
    """Build + compile the per-core Bass program (SPMD, identical on all
    cores).

    DRAM inputs (per core):
      xt [KT, 128, rows_c]  bf16 : x-shard transposed, k-tiled
      w  [G, 128, JG*KT*128] bf16 : w_int, G groups of JG j-tiles;
            w[g, p, jtl*KT*128 + kt*128 + f] = w_int[kt*128+p, (g*JG+jtl)*128+f]
            so each group is one [128 x JG*KT*128] 2D DMA (16KB/partition).
      bt [128, JT]          f32  : bias, bt[p, jt] = b[jt*128 + p]
    DRAM output:
      o  [JT, 128, rows_c]  f32  : outT tiles, o[jt, p, i] = outT[jt*128+p, i]
    """
    KT = d_in // P
    JT = units // P
    NS = rows_c // N_SLICE
    # mm_dtype: "bfloat16" | "float32r" | "mixed" (bf16 weights, f32r x)
    w_dt = mybir.dt.bfloat16 if mm_dtype in ("bfloat16", "mixed") \
        else mybir.dt.float32r
    x_dt = mybir.dt.bfloat16 if mm_dtype == "bfloat16" else mybir.dt.float32r
    # j-tiles per weight DMA group (f32r tiles are 2x the bytes; keep SBUF fit)
    JG = min(8 if w_dt == mybir.dt.bfloat16 else 4, JT)
    G = JT // JG
    WBUFS = 3 if w_dt == mybir.dt.bfloat16 else 2

    nc = bacc.Bacc(None, target_bir_lowering=False)
    xt = nc.dram_tensor("xt", [KT, P, rows_c], x_dt, kind="ExternalInput")
    w = nc.dram_tensor("w", [G, P, JG * KT * P], w_dt, kind="ExternalInput")
    bt = nc.dram_tensor("bt", [P, JT], mybir.dt.float32, kind="ExternalInput")
    o = nc.dram_tensor("o", [JT, P, rows_c], mybir.dt.float32,
                       kind="ExternalOutput")

    with tile.TileContext(nc) as tc:
        with (
            tc.tile_pool(name="xpool", bufs=1) as xpool,
            tc.tile_pool(name="bpool", bufs=1) as bpool,
            tc.tile_pool(name="wpool", bufs=WBUFS) as wpool,
            tc.tile_pool(name="opool", bufs=3) as opool,
            tc.tile_pool(name="pspool", bufs=4, space="PSUM") as pspool,
        ):
            # Prologue: w group 0 lands piecewise on the SP HW queue so the
            # first j-tile's weights arrive early, while x k-tiles stream in
            # parallel on the Activation engine's HW queue. The PE starts as
            # soon as w[jtl=0] + x[kt=0] are in (slice-level deps).
            wsbs = [wpool.tile([P, JG * KT * P], w_dt,
                               tag="wsb", name=f"wsb{g}") for g in range(G)]
            nc.sync.dma_start(out=wsbs[0][:, :KT * P], in_=w[0, :, :KT * P])

            # Resident activations: all k-tiles of xT, side by side.
            # (Splitting these into n-halves for an earlier first psum group
            # was tried and measured ~8 us slower: the half-tile reads are
            # 2 KB strided segments vs 4 KB contiguous rows.)
            xsb = xpool.tile([P, KT * rows_c], x_dt)
            for kt in range(KT):
                nc.scalar.dma_start(
                    out=xsb[:, kt * rows_c:(kt + 1) * rows_c], in_=xt[kt]
                )
            bsb = bpool.tile([P, JT], mybir.dt.float32)
            nc.scalar.dma_start(out=bsb[:, :], in_=bt[:, :])
            # g0's remaining j-tiles ride the ACT queue behind x (needed
            # only ~25us in); keeps the first MM's SP-queue wait to jtl0.
            for jtl in range(1, JG):
                nc.scalar.dma_start(
                    out=wsbs[0][:, jtl * KT * P:(jtl + 1) * KT * P],
                    in_=w[0, :, jtl * KT * P:(jtl + 1) * KT * P],
                )

            for g in range(G):
                wsb = wsbs[g]
                if g > 0:
                    # Prefetch on the Activation HW queue, queued behind the
                    # x tiles: keeps the early weight groups from stealing
                    # HBM bandwidth from the critical x stream, while the SP
                    # queue carries only w group 0 + output writes.
                    nc.scalar.dma_start(out=wsb[:, :], in_=w[g])
                for jtl in range(JG):
                    jt = g * JG + jtl
                    ob = opool.tile([P, rows_c], mybir.dt.float32)
                    for n in range(NS):
                        ps = pspool.tile([P, N_SLICE], mybir.dt.float32)
                        for kt in range(KT):
                            wof = jtl * KT * P + kt * P
                            nc.tensor.matmul(
                                ps[:, :],
                                wsb[:, wof:wof + P],
                                xsb[:, kt * rows_c + n * N_SLICE:
                                       kt * rows_c + (n + 1) * N_SLICE],
                                start=(kt == 0),
                                stop=(kt == KT - 1),
                            )
                        nc.scalar.activation(
                            ob[:, n * N_SLICE:(n + 1) * N_SLICE],
                            ps[:, :],
                            mybir.ActivationFunctionType.Relu,
                            bias=bsb[:, jt:jt + 1],
                            scale=float(scale),
                        )
                    nc.sync.dma_start(out=o[jt], in_=ob[:, :])

    nc.compile()
    return nc


def prep_w(w_int, d_in: int = None, units: int = None,
           mm_dtype: str = "bfloat16"):
    """[d_in, units] -> [G, 128, JG*KT*128]; see build_nc docstring."""
    d_in = d_in or w_int.shape[0]
    units = units or w_int.shape[1]
    KT, JT = d_in // P, units // P
    JG = min(8 if mm_dtype in ("bfloat16", "mixed") else 4, JT)
    G = JT // JG
    return np.ascontiguousarray(
        w_int.reshape(KT, P, G, JG, P)        # [kt, p, g, jtl, f]
             .transpose(2, 1, 3, 0, 4)        # [g, p, jtl, kt, f]
             .reshape(G, P, JG * KT * P)
    )


_NC_CACHE: dict = {}


MM_DTYPE = "float32r"   # "bfloat16" | "float32r" | "mixed"


def _get_nc(scale: float):
    key = (round(float(scale), 12), MM_DTYPE)
    if key not in _NC_CACHE:
        _NC_CACHE[key] = build_nc(float(scale), mm_dtype=MM_DTYPE)
    return _NC_CACHE[key]


def kernel(inputs, w_quantized, quantized_scale, zero_point, b):
    scale = float(np.asarray(quantized_scale))
    zp = float(np.asarray(zero_point))

    # Exact integer weights in bf16 (w - zp with zp = -3.0 stays a small
    # integer; bf16 represents integers up to 256 exactly).
    w_int = (np.asarray(w_quantized).astype(np.float32) - zp)
    if MM_DTYPE in ("bfloat16", "mixed"):
        w_int = w_int.astype(BF16)
    w_tiled = prep_w(w_int, mm_dtype=MM_DTYPE)

    bt = np.ascontiguousarray(
        np.asarray(b).astype(np.float32).reshape(UNITS // P, P).T
    )

    x_bf = np.asarray(inputs).astype(np.float32)
    if MM_DTYPE == "bfloat16":
        x_bf = x_bf.astype(BF16)

    in_maps = []
    for c in range(N_CORES):
        shard = x_bf[c * ROWS_C:(c + 1) * ROWS_C, :]          # [1024, 2048]
        xt_c = np.ascontiguousarray(shard.T).reshape(D_IN // P, P, ROWS_C)
        in_maps.append({"xt": xt_c, "w": w_tiled, "bt": bt})

    nc = _get_nc(scale)
    results = run_bass_kernel_spmd(nc, in_maps, core_ids=list(range(N_CORES)))
    global _LAST_RESULTS
    _LAST_RESULTS = results

    out = np.empty((ROWS, UNITS), dtype=np.float32)
    for c in range(N_CORES):
        outT = results.results[c]["o"].reshape(UNITS, ROWS_C)
        out[c * ROWS_C:(c + 1) * ROWS_C, :] = outT.T
    return out



# revision 2
# speedup vs baseline: 1.0895x; 1.0895x over previous
"""Quantized dense MLP kernel for 8 Trainium2 NeuronCores.

Problem: out = relu(inputs @ ((w_int8 - zero_point) * scale) + b)
  inputs [8192, 2048] f32, w_quantized [2048, 8192] int8,
  scale/zero_point f32 scalars, b [8192] f32 -> out [8192, 8192] f32.

Strategy:
- Data-parallel: shard rows of `inputs` across 8 cores (1024 rows each).
- Zero-point folding: w_int = w_int8 - zero_point (zero_point = -3.0) is a
  small integer, exactly representable in bf16/f32. The scale and bias are
  applied on the ScalarEngine in f32: out = Relu(scale * acc + b).
- The PE matmul runs in float32r (TF32-like, full bf16 throughput at
  N=512): exact weights, ~1e-4 rel err from the reduced-precision x
  streaming. (MM_DTYPE="bfloat16" variant: ~6% faster, ~1.7e-3 rel err.)
- On device each core computes outT[j, i] = sum_k w_int[k, j] * xT[k, i]
  (w tile stationary, xT moving), so the bias b_j is a natural
  per-partition activation bias. The host transposes each core's outT
  back and stacks.
- Per core: x^T stays SBUF-resident; weights stream as G large 2D DMAs
  (the sync engine pays ~600 ns issue per DMA regardless of size, so few
  big transfers); group 0 lands piecewise with x on a parallel HW queue
  so the PE starts within ~16 us.
"""

import numpy as np
import ml_dtypes

import concourse.bass as bass
import concourse.mybir as mybir
import concourse.tile as tile
from concourse import bacc
from concourse.bass_utils import run_bass_kernel_spmd

BF16 = ml_dtypes.bfloat16

# Full problem dims (hardcoded per harness contract).
ROWS, D_IN, UNITS = 8192, 2048, 8192
N_CORES = 8
ROWS_C = ROWS // N_CORES  # rows per core

P = 128         # SBUF partitions
N_SLICE = 512   # moving free dim per matmul (one PSUM bank of f32)


def build_nc(scale: float, d_in: int = D_IN, units: int = UNITS,
             rows_c: int = ROWS_C, mm_dtype: str = "bfloat16"):
    """Build + compile the per-core Bass program (SPMD, identical on all
    cores).

    DRAM inputs (per core):
      xt [KT, 128, rows_c]  bf16 : x-shard transposed, k-tiled
      w  [G, 128, JG*KT*128] bf16 : w_int, G groups of JG j-tiles;
            w[g, p, jtl*KT*128 + kt*128 + f] = w_int[kt*128+p, (g*JG+jtl)*128+f]
            so each group is one [128 x JG*KT*128] 2D DMA (16KB/partition).
      bt [128, JT]          f32  : bias, bt[p, jt] = b[jt*128 + p]
    DRAM output:
      o  [JT, 128, rows_c]  f32  : outT tiles, o[jt, p, i] = outT[jt*128+p, i]
    """
    KT = d_in // P
    JT = units // P
    NS = rows_c // N_SLICE
    # mm_dtype: "bfloat16" | "float32r" | "mixed" (bf16 weights, f32r x)
    w_dt = mybir.dt.bfloat16 if mm_dtype in ("bfloat16", "mixed") \
        else mybir.dt.float32r
    x_dt = mybir.dt.bfloat16 if mm_dtype == "bfloat16" else mybir.dt.float32r
    # j-tiles per weight DMA group (f32r tiles are 2x the bytes; keep SBUF fit)
    JG = min(8 if w_dt == mybir.dt.bfloat16 else 4, JT)
    G = JT // JG
    WBUFS = 3 if w_dt == mybir.dt.bfloat16 else 2

    nc = bacc.Bacc(None, target_bir_lowering=False)
    xt = nc.dram_tensor("xt", [KT, P, rows_c], x_dt, kind="ExternalInput")
    w = nc.dram_tensor("w", [G, P, JG * KT * P], w_dt, kind="ExternalInput")
    bt = nc.dram_tensor("bt", [P, JT], mybir.dt.float32, kind="ExternalInput")
    o = nc.dram_tensor("o", [JT, P, rows_c], mybir.dt.float32,
                       kind="ExternalOutput")

    with tile.TileContext(nc) as tc:
        with (
            tc.tile_pool(name="xpool", bufs=1) as xpool,
            tc.tile_pool(name="bpool", bufs=1) as bpool,
            tc.tile_pool(name="wpool", bufs=WBUFS) as wpool,
            tc.tile_pool(name="opool", bufs=3) as opool,
            tc.tile_pool(name="pspool", bufs=4, space="PSUM") as pspool,
        ):
            # Prologue: w group 0 lands piecewise on the SP HW queue so the
            # first j-tile's weights arrive early, while x k-tiles stream in
            # parallel on the Activation engine's HW queue. The PE starts as
            # soon as w[jtl=0] + x[kt=0] are in (slice-level deps).
            wsbs = [wpool.tile([P, JG * KT * P], w_dt,
                               tag="wsb", name=f"wsb{g}") for g in range(G)]
            nc.sync.dma_start(out=wsbs[0][:, :KT * P], in_=w[0, :, :KT * P])

            # Resident activations: all k-tiles of xT, side by side.
            # (Splitting these into n-halves for an earlier first psum group
            # was tried and measured ~8 us slower: the half-tile reads are
            # 2 KB strided segments vs 4 KB contiguous rows.)
            xsb = xpool.tile([P, KT * rows_c], x_dt)
            for kt in range(KT):
                nc.scalar.dma_start(
                    out=xsb[:, kt * rows_c:(kt + 1) * rows_c], in_=xt[kt]
                )
            bsb = bpool.tile([P, JT], mybir.dt.float32)
            nc.scalar.dma_start(out=bsb[:, :], in_=bt[:, :])
            # g0's remaining j-tiles ride the ACT queue behind x (needed
            # only ~25us in); keeps the first MM's SP-queue wait to jtl0.
            for jtl in range(1, JG):
                nc.scalar.dma_start(
                    out=wsbs[0][:, jtl * KT * P:(jtl + 1) * KT * P],
                    in_=w[0, :, jtl * KT * P:(jtl + 1) * KT * P],
                )

            for g in range(G):
                wsb = wsbs[g]
                if g > 0:
                    # Prefetch on the Activation HW queue, queued behind the
                    # x tiles: keeps the early weight groups from stealing
                    # HBM bandwidth from the critical x stream, while the SP
                    # queue carries only w group 0 + output writes.
                    nc.scalar.dma_start(out=wsb[:, :], in_=w[g])
                for jtl in range(JG):
                    jt = g * JG + jtl
                    ob = opool.tile([P, rows_c], mybir.dt.float32)
                    for n in range(NS):
                        ps = pspool.tile([P, N_SLICE], mybir.dt.float32)
                        for kt in range(KT):
                            wof = jtl * KT * P + kt * P
                            nc.tensor.matmul(
                                ps[:, :],
                                wsb[:, wof:wof + P],
                                xsb[:, kt * rows_c + n * N_SLICE:
                                       kt * rows_c + (n + 1) * N_SLICE],
                                start=(kt == 0),
                                stop=(kt == KT - 1),
                            )
                        nc.scalar.activation(
                            ob[:, n * N_SLICE:(n + 1) * N_SLICE],
                            ps[:, :],
                            mybir.ActivationFunctionType.Relu,
                            bias=bsb[:, jt:jt + 1],
                            scale=float(scale),
                        )
                    nc.sync.dma_start(out=o[jt], in_=ob[:, :])

    nc.compile()
    return nc


def prep_w(w_int, d_in: int = None, units: int = None,
           mm_dtype: str = "bfloat16"):
    """[d_in, units] -> [G, 128, JG*KT*128]; see build_nc docstring."""
    d_in = d_in or w_int.shape[0]
    units = units or w_int.shape[1]
    KT, JT = d_in // P, units // P
    JG = min(8 if mm_dtype in ("bfloat16", "mixed") else 4, JT)
    G = JT // JG
    return np.ascontiguousarray(
        w_int.reshape(KT, P, G, JG, P)        # [kt, p, g, jtl, f]
             .transpose(2, 1, 3, 0, 4)        # [g, p, jtl, kt, f]
             .reshape(G, P, JG * KT * P)
    )


_NC_CACHE: dict = {}


MM_DTYPE = "bfloat16"   # "bfloat16" | "float32r" | "mixed"


def _get_nc(scale: float):
    key = (round(float(scale), 12), MM_DTYPE)
    if key not in _NC_CACHE:
        _NC_CACHE[key] = build_nc(float(scale), mm_dtype=MM_DTYPE)
    return _NC_CACHE[key]


def kernel(inputs, w_quantized, quantized_scale, zero_point, b):
    scale = float(np.asarray(quantized_scale))
    zp = float(np.asarray(zero_point))

    # Exact integer weights in bf16 (w - zp with zp = -3.0 stays a small
    # integer; bf16 represents integers up to 256 exactly).
    w_int = (np.asarray(w_quantized).astype(np.float32) - zp)
    if MM_DTYPE in ("bfloat16", "mixed"):
        w_int = w_int.astype(BF16)
    w_tiled = prep_w(w_int, mm_dtype=MM_DTYPE)

    bt = np.ascontiguousarray(
        np.asarray(b).astype(np.float32).reshape(UNITS // P, P).T
    )

    x_bf = np.asarray(inputs).astype(np.float32)
    if MM_DTYPE == "bfloat16":
        x_bf = x_bf.astype(BF16)

    in_maps = []
    for c in range(N_CORES):
        shard = x_bf[c * ROWS_C:(c + 1) * ROWS_C, :]          # [1024, 2048]
        xt_c = np.ascontiguousarray(shard.T).reshape(D_IN // P, P, ROWS_C)
        in_maps.append({"xt": xt_c, "w": w_tiled, "bt": bt})

    nc = _get_nc(scale)
    results = run_bass_kernel_spmd(nc, in_maps, core_ids=list(range(N_CORES)))
    global _LAST_RESULTS
    _LAST_RESULTS = results

    out = np.empty((ROWS, UNITS), dtype=np.float32)
    for c in range(N_CORES):
        outT = results.results[c]["o"].reshape(UNITS, ROWS_C)
        out[c * ROWS_C:(c + 1) * ROWS_C, :] = outT.T
    return out



# revision 6
# speedup vs baseline: 1.1596x; 1.0643x over previous
"""Quantized dense MLP kernel for 8 Trainium2 NeuronCores.

Problem: out = relu(inputs @ ((w_int8 - zero_point) * scale) + b)
  inputs [8192, 2048] f32, w_quantized [2048, 8192] int8,
  scale/zero_point f32 scalars, b [8192] f32 -> out [8192, 8192] f32.

Strategy:
- Data-parallel: shard rows of `inputs` across 8 cores (1024 rows each).
- Zero-point folding: w_int = w_int8 - zero_point (zero_point = -3.0) is a
  small integer, exactly representable in bf16. Scale and bias are applied
  on the ScalarEngine in f32: out = Relu(scale * acc + b).
- Hybrid precision contraction (per 512-col psum group):
  * k-tiles 0..13 (k < 1792): bf16 matmuls — weights exact, x rounded to
    bf16 (the only bf16 error source, ~1.7e-3 rel).
  * k-tiles 14,15 (k 1792..2047): ONE fp8e4m3 DoubleRow matmul carrying
    both k-tiles (K_eff=256) in the same 216 ns a bf16 matmul takes.
    e4m3 quantization of x and w on 2/16 of the sum adds ~1.26e-2 rel
    error; total measured ~1.28e-2 (threshold 2e-2).
  This makes 15 matmul instructions per group instead of 16 (-6.25% PE).
- Prologue: the first two j-tiles' four psum groups are interleaved
  kt-major so the PE has runnable matmuls while x streams in; x k-tile
  DMAs are split across the SP and ACT hardware queues; the first weight
  j-tiles land as small chunks so the first matmul issues at ~8 us.
- Outputs are written per 512-row half right after each activation to
  shorten the drain tail.
"""

import sys
import types

import numpy as np
import ml_dtypes

import concourse.bass as bass
import concourse.mybir as mybir
import concourse.tile as tile
from concourse import bacc
from concourse.bass_utils import run_bass_kernel_spmd

# If BASS_TRACE is set but this image's `antenv` lacks `axon_hooks`,
# bass_utils would crash importing it. Provide a stub that reports "no
# hook registered" so tracing degrades gracefully instead.
try:
    import antenv

    if not hasattr(antenv, "axon_hooks"):
        _ah = types.ModuleType("antenv.axon_hooks")
        _ah._hook = None
        _ah.set_axon_ntff_profile_hook = lambda h, _m=_ah: setattr(_m, "_hook", h)
        _ah.get_axon_ntff_profile_hook = lambda _m=_ah: _m._hook
        sys.modules["antenv.axon_hooks"] = _ah
        antenv.axon_hooks = _ah
        try:
            from trn_agent_boot.trn_boot import _ntff_profile_via_ctypes

            _ah.set_axon_ntff_profile_hook(
                _ntff_profile_via_ctypes("/opt/axon/libaxon_pjrt.so"))
        except Exception:
            pass
except Exception:
    pass

BF16 = ml_dtypes.bfloat16
E4M3 = ml_dtypes.float8_e4m3

# Full problem dims (hardcoded per harness contract).
ROWS, D_IN, UNITS = 8192, 2048, 8192
N_CORES = 8
ROWS_C = ROWS // N_CORES  # rows per core

P = 128         # SBUF partitions
N_SLICE = 512   # moving free dim per matmul (one PSUM bank of f32)
KT = D_IN // P            # 16 k-tiles
KTB = KT - 2              # 14 k-tiles in bf16; last 2 ride one fp8 DR matmul
JT = UNITS // P           # 64 j-tiles
JG = 8                    # j-tiles per weight DMA group
G = JT // JG              # 8 groups
NS = ROWS_C // N_SLICE    # 2 n-slices
JT_PRE = 2                # j-tiles interleaved kt-major in the prologue


def build_nc(scale: float):
    """Build + compile the per-core Bass program (SPMD, identical cores).

    DRAM inputs (per core):
      xt [KTB, 128, ROWS_C] bf16 : x-shard transposed, k-tiled (kt 0..13)
      x8 [128, 2, ROWS_C]   f8e4 : x-shard k rows 1792..2047, slot s=kt-14
      w  [G, 128, JG*KTB*128] bf16 : w_int, per g: [jtl][kt][j]
      w8 [G, 128, JG*2*128]   f8e4 : e4m3(w_int) k rows 1792+, [jtl][s][j]
      bt [128, JT]          f32  : bias, bt[p, jt] = b[jt*128 + p]
    DRAM output:
      o  [JT, 128, ROWS_C]  f32  : outT tiles, o[jt, p, i] = outT[jt*128+p, i]
    """
    DR = mybir.MatmulPerfMode.DoubleRow
    nc = bacc.Bacc(None, target_bir_lowering=False)
    xt = nc.dram_tensor("xt", [KTB, P, ROWS_C], mybir.dt.bfloat16,
                        kind="ExternalInput")
    x8 = nc.dram_tensor("x8", [P, 2, ROWS_C], mybir.dt.float8e4,
                        kind="ExternalInput")
    w = nc.dram_tensor("w", [G, P, JG * KTB * P], mybir.dt.bfloat16,
                       kind="ExternalInput")
    w8 = nc.dram_tensor("w8", [G, P, JG * 2 * P], mybir.dt.float8e4,
                        kind="ExternalInput")
    bt = nc.dram_tensor("bt", [P, JT], mybir.dt.float32, kind="ExternalInput")
    o = nc.dram_tensor("o", [JT, P, ROWS_C], mybir.dt.float32,
                       kind="ExternalOutput")

    with tile.TileContext(nc) as tc:
        with (
            tc.tile_pool(name="xpool", bufs=1) as xpool,
            tc.tile_pool(name="bpool", bufs=1) as bpool,
            tc.tile_pool(name="wpool", bufs=3) as wpool,
            tc.tile_pool(name="w8pool", bufs=3) as w8pool,
            tc.tile_pool(name="opool", bufs=3) as opool,
            tc.tile_pool(name="pspool", bufs=4, space="PSUM") as pspool,
        ):
            wsbs = [wpool.tile([P, JG, KTB, P], mybir.dt.bfloat16,
                               tag="wsb", name=f"wsb{g}") for g in range(G)]
            w8sbs = [w8pool.tile([P, JG, 2, P], mybir.dt.float8e4,
                                 tag="w8sb", name=f"w8sb{g}") for g in range(G)]
            xsb = xpool.tile([P, KTB, ROWS_C], mybir.dt.bfloat16)
            x8sb = xpool.tile([P, 2, ROWS_C], mybir.dt.float8e4)
            bsb = bpool.tile([P, JT], mybir.dt.float32)

            # --- prologue DMAs -------------------------------------------
            # SP queue: jtl0 weights first (chunks so the PE starts early),
            # then the fp8 tail weights for jtl0/1, bias, jtl1 chunks, and
            # the odd x k-tiles. ACT queue: even x k-tiles, then the rest
            # of group 0 and all prefetched groups.
            H = KTB // 2  # 7-k-tile weight chunks
            nc.sync.dma_start(out=wsbs[0][:, 0, 0:H, :],
                              in_=w[0][:, 0:H * P])
            nc.sync.dma_start(out=w8sbs[0][:, 0:2, :, :],
                              in_=w8[0][:, 0:2 * 2 * P])
            nc.sync.dma_start(out=bsb[:, :], in_=bt[:, :])
            nc.scalar.dma_start(out=x8sb[:, :, :], in_=x8[:, :, :])
            for kt in range(0, KTB, 2):  # even k-tiles on ACT
                nc.scalar.dma_start(out=xsb[:, kt, :], in_=xt[kt])
            nc.sync.dma_start(out=wsbs[0][:, 1, 0:H, :],
                              in_=w[0][:, (KTB + 0) * P:(KTB + H) * P])
            nc.sync.dma_start(out=xsb[:, 1, :], in_=xt[1])
            nc.sync.dma_start(out=xsb[:, 3, :], in_=xt[3])
            nc.sync.dma_start(out=wsbs[0][:, 0, H:KTB, :],
                              in_=w[0][:, H * P:KTB * P])
            nc.sync.dma_start(out=wsbs[0][:, 1, H:KTB, :],
                              in_=w[0][:, (KTB + H) * P:2 * KTB * P])
            for kt in range(5, KTB, 2):  # remaining odd k-tiles
                nc.sync.dma_start(out=xsb[:, kt, :], in_=xt[kt])
            # rest of group 0 on ACT, behind the even x k-tiles
            for jtl in range(JT_PRE, JG):
                nc.scalar.dma_start(
                    out=wsbs[0][:, jtl, :, :],
                    in_=w[0][:, jtl * KTB * P:(jtl + 1) * KTB * P])
            nc.scalar.dma_start(out=w8sbs[0][:, JT_PRE:, :, :],
                                in_=w8[0][:, JT_PRE * 2 * P:])

            def mm_group(ps, g, jtl, n, kt_range, start, stop):
                wsb, w8sb = wsbs[g], w8sbs[g]
                sl = slice(n * N_SLICE, (n + 1) * N_SLICE)
                for kt in kt_range:
                    nc.tensor.matmul(
                        ps[:, :], wsb[:, jtl, kt, :], xsb[:, kt, sl],
                        start=(start and kt == 0), stop=False)
                if stop:
                    nc.tensor.matmul(
                        ps[:, :], w8sb[:, jtl, :, :], x8sb[:, :, sl],
                        start=False, stop=True, perf_mode=DR)

            def act_and_store(ps, ob, jt, n):
                sl = slice(n * N_SLICE, (n + 1) * N_SLICE)
                nc.scalar.activation(
                    ob[:, sl], ps[:, :],
                    mybir.ActivationFunctionType.Relu,
                    bias=bsb[:, jt:jt + 1], scale=float(scale))
                nc.sync.dma_start(out=o[jt][:, sl], in_=ob[:, sl])

            # --- phase 1: jt 0..1 interleaved kt-major -------------------
            pre_ps = [pspool.tile([P, N_SLICE], mybir.dt.float32,
                                  tag="ps", name=f"pre_ps{i}")
                      for i in range(JT_PRE * NS)]
            pre_ob = [opool.tile([P, ROWS_C], mybir.dt.float32,
                                 tag="ob", name=f"pre_ob{i}")
                      for i in range(JT_PRE)]
            for kt in range(KTB):
                for jtl in range(JT_PRE):
                    for n in range(NS):
                        ps = pre_ps[jtl * NS + n]
                        nc.tensor.matmul(
                            ps[:, :], wsbs[0][:, jtl, kt, :],
                            xsb[:, kt, n * N_SLICE:(n + 1) * N_SLICE],
                            start=(kt == 0), stop=False)
            for jtl in range(JT_PRE):
                for n in range(NS):
                    ps = pre_ps[jtl * NS + n]
                    nc.tensor.matmul(
                        ps[:, :], w8sbs[0][:, jtl, :, :],
                        x8sb[:, :, n * N_SLICE:(n + 1) * N_SLICE],
                        start=False, stop=True, perf_mode=DR)
                    act_and_store(ps, pre_ob[jtl], jtl, n)

            # --- phase 2: jt 2..63, n-major ------------------------------
            for g in range(G):
                if g > 0:
                    nc.scalar.dma_start(out=wsbs[g][:, :, :, :], in_=w[g])
                    nc.scalar.dma_start(out=w8sbs[g][:, :, :, :], in_=w8[g])
                for jtl in range(JT_PRE if g == 0 else 0, JG):
                    jt = g * JG + jtl
                    ob = opool.tile([P, ROWS_C], mybir.dt.float32,
                                    tag="ob")
                    for n in range(NS):
                        ps = pspool.tile([P, N_SLICE], mybir.dt.float32,
                                         tag="ps")
                        mm_group(ps, g, jtl, n, range(KTB), True, True)
                        act_and_store(ps, ob, jt, n)

    nc.compile()
    return nc


_NC_CACHE: dict = {}


def _get_nc(scale: float):
    key = round(float(scale), 12)
    if key not in _NC_CACHE:
        _NC_CACHE[key] = build_nc(float(scale))
    return _NC_CACHE[key]


def kernel(inputs, w_quantized, quantized_scale, zero_point, b):
    scale = float(np.asarray(quantized_scale))
    zp = float(np.asarray(zero_point))
    KB = KTB * P  # bf16 k-range boundary (1792)

    # Exact integer weights (w - zp with zp = -3.0 stays a small integer;
    # bf16 represents integers up to 256 exactly). The last 2 k-tiles are
    # quantized to e4m3 for the DoubleRow tail matmul.
    w_int = np.asarray(w_quantized).astype(np.float32) - zp
    wb = np.ascontiguousarray(
        w_int[:KB, :].astype(BF16)
             .reshape(KTB, P, G, JG, P)      # [kt, p, g, jtl, j]
             .transpose(2, 1, 3, 0, 4)       # [g, p, jtl, kt, j]
             .reshape(G, P, JG * KTB * P))
    w8 = np.ascontiguousarray(
        w_int[KB:, :].astype(E4M3)
             .reshape(2, P, G, JG, P)        # [s, p, g, jtl, j]
             .transpose(2, 1, 3, 0, 4)       # [g, p, jtl, s, j]
             .reshape(G, P, JG * 2 * P))

    bt = np.ascontiguousarray(
        np.asarray(b).astype(np.float32).reshape(JT, P).T)

    x_f32 = np.asarray(inputs).astype(np.float32)

    in_maps = []
    for c in range(N_CORES):
        shard = x_f32[c * ROWS_C:(c + 1) * ROWS_C, :]          # [1024, 2048]
        xt_c = np.ascontiguousarray(
            shard[:, :KB].astype(BF16).T.reshape(KTB, P, ROWS_C))
        x8_c = np.ascontiguousarray(
            shard[:, KB:].astype(E4M3).T.reshape(2, P, ROWS_C)
                 .transpose(1, 0, 2))                          # [P, 2, ROWS_C]
        in_maps.append({"xt": xt_c, "x8": x8_c, "w": wb, "w8": w8, "bt": bt})

    nc = _get_nc(scale)
    results = run_bass_kernel_spmd(nc, in_maps, core_ids=list(range(N_CORES)))
    global _LAST_RESULTS
    _LAST_RESULTS = results

    out = np.empty((ROWS, UNITS), dtype=np.float32)
    for c in range(N_CORES):
        outT = results.results[c]["o"].reshape(UNITS, ROWS_C)
        out[c * ROWS_C:(c + 1) * ROWS_C, :] = outT.T
    return out
